# revision 4
# baseline (speedup 1.0000x reference)
"""LIF spike kernel for Trainium2 (Bass/Tile), data-parallel over batch on 8 cores.

Host layout per core: x_core [C=128, T=8, M=B_loc*HW=4096] f32, so each
timestep is one [128, 4096] tile (16 KB per partition, 2 MB per DMA).

State is u_t (pre-reset membrane); the hard reset folds into the next step:
  u_t     = select(u_{t-1} <= 1, u_{t-1}, 0) * 0.5 + x_t   (custom DVE op, 1 op/step)
  spike_t = sign(u_t - 1) saturated to uint8               (ACT engine, exact {0,1})

sign->u8 saturates negatives to 0 (HW-verified), so spike == (u > 1) exactly,
including u == 1 (sign(0) = 0). All arithmetic is bit-exact fp32 vs the
reference (mult by 0.5 exact, one rounding add, select exact).

Input DMAs are issued on the sync HWDGE ring before the output DMAs so the
FIFO drains all of x first; outputs (ready early) don't steal read bandwidth.
"""

import numpy as np

import concourse.bacc as bacc
import concourse.mybir as mybir
import concourse.dve_ops as dve_ops
from concourse.dve_ops import DveOp
from concourse.dve_spec import Spec, Src0, Src1, C0, C1, Zero, select, lower, _has_src1
from concourse.dve_uop import DveOpSpec
from concourse.dve_table_gen import dve_ver_for
from concourse.tile import TileContext
from concourse.bass_utils import run_bass_kernel_spmd

B, T, C, H, W = 32, 8, 128, 32, 32
HW = H * W
N_CORES = 8
B_LOC = B // N_CORES
M = B_LOC * HW  # 4096
TAU = 0.5
THRESH = 1.0

_nc_cache = None


def _register_lif_step():
    name = "LIF_STEP_ANT"
    for op in dve_ops.OPS:
        if op.name == name:
            return op

    def _ref(in0, in1, s0, s1, imm2):
        m = np.where(
            np.asarray(in0, np.float32) <= np.float32(s1), in0, np.float32(0.0)
        ).astype(np.float32)
        return (m * np.float32(s0) + np.asarray(in1, np.float32)).astype(np.float32)

    body = select(Src0 <= C1, Src0, Zero) * C0 + Src1
    spec = Spec(body=body, reference=_ref)
    row = dve_ops._CUSTOM_DVE_ROW_BASE + len(dve_ops.OPS)
    ver = dve_ver_for("TRN2")
    tmp = DveOpSpec(name=name, opcode=row, uops=lower(spec, ver=ver), rd1_en=_has_src1(spec))
    op = DveOp(name, spec, subdim=False, uops_sha={ver: tmp.sha(ver)})
    dve_ops.OPS.append(op)
    dve_ops._SUB_OPCODE_FOR_NAME[name] = row
    dve_ops.CUSTOM_DVE_SPECS[name] = spec
    return op


def build_nc():
    lif_op = _register_lif_step()
    nc = bacc.Bacc("TRN2", target_bir_lowering=False)
    f32 = mybir.dt.float32
    u8 = mybir.dt.uint8
    act = mybir.ActivationFunctionType

    x = nc.dram_tensor("x", [C, T, M], f32, kind="ExternalInput")
    out = nc.dram_tensor("out", [C, T, M], u8, kind="ExternalOutput")

    with TileContext(nc) as tc:
        with (
            tc.tile_pool(name="xp", bufs=T) as xp,
            tc.tile_pool(name="up", bufs=3) as up,
            tc.tile_pool(name="sp", bufs=3) as sp,
            tc.tile_pool(name="cp", bufs=1) as cp,
        ):
            negone = cp.tile([C, 1], f32, tag="negone")
            nc.gpsimd.memset(negone[:], -1.0)

            xt = []
            for t in range(T):
                tile = xp.tile([C, M], f32, tag="x")
                # alternate the two HWDGE rings so per-transfer completion
                # tails overlap instead of serializing on one FIFO
                eng = nc.sync if t % 2 == 0 else nc.scalar
                eng.dma_start(out=tile[:], in_=x[:, t, :])
                xt.append(tile)

            u_prev = None
            for t in range(T):
                if t == 0:
                    u = xt[0]
                else:
                    u = up.tile([C, M], f32, tag="u")
                    nc.vector._custom_dve(
                        lif_op, out=u[:], in0=u_prev[:], in1=xt[t][:],
                        s0=TAU, s1=THRESH,
                    )
                s = sp.tile([C, M], u8, tag="s")
                nc.scalar.activation(s[:], u[:], act.Sign, bias=negone[:])
                # outputs alternate rings like the inputs (4+4); each ring's
                # FIFO drains its outputs after its inputs, so reads keep
                # full bandwidth and the rings stay balanced (10 MB each)
                eng = nc.sync if t % 2 == 0 else nc.scalar
                eng.dma_start(out=out[:, t, :], in_=s[:])
                u_prev = u
    nc.compile()
    return nc


def make_in_maps(x: np.ndarray) -> list[dict]:
    # x [B,T,C,H,W] -> per core [C, T, B_loc*HW]
    xs = np.ascontiguousarray(x).reshape(B, T, C, HW)
    return [
        {
            "x": np.ascontiguousarray(
                xs[i * B_LOC : (i + 1) * B_LOC].transpose(2, 1, 0, 3)
            ).reshape(C, T, M)
        }
        for i in range(N_CORES)
    ]


def kernel(x: np.ndarray) -> np.ndarray:
    global _nc_cache
    if _nc_cache is None:
        _nc_cache = build_nc()
    res = run_bass_kernel_spmd(_nc_cache, make_in_maps(x), list(range(N_CORES)))
    # out[c, t, b_loc*HW] -> [b, t, c, hw]
    parts = [
        res.results[i]["out"].reshape(C, T, B_LOC, HW).transpose(2, 1, 0, 3)
        for i in range(N_CORES)
    ]
    full = np.concatenate(parts, axis=0)
    return full.reshape(B, T, C, H, W).astype(np.float32)


# revision 5
# speedup vs baseline: 1.1870x; 1.1870x over previous
"""LIF spike kernel for Trainium2 (Bass/Tile), data-parallel over batch on 8 cores.

Host layout per core: x_core [C=128, T=8, M=B_loc*HW=4096] f32, so each
timestep is one [128, 4096] tile (16 KB per partition, 2 MB per DMA).

State is u_t (pre-reset membrane); the hard reset folds into the next step:
  u_t     = select(u_{t-1} <= 1, u_{t-1}, 0) * 0.5 + x_t   (custom DVE op, 1 op/step)
  spike_t = sign(u_t - 1) saturated to uint8               (ACT engine, exact {0,1})

sign->u8 saturates negatives to 0 (HW-verified), so spike == (u > 1) exactly,
including u == 1 (sign(0) = 0). All arithmetic is bit-exact fp32 vs the
reference (mult by 0.5 exact, one rounding add, select exact).

Input DMAs are issued on the sync HWDGE ring before the output DMAs so the
FIFO drains all of x first; outputs (ready early) don't steal read bandwidth.
"""

import numpy as np

import concourse.bacc as bacc
import concourse.mybir as mybir
import concourse.dve_ops as dve_ops
from concourse.dve_ops import DveOp
from concourse.dve_spec import Spec, Src0, Src1, C0, C1, Zero, select, lower, _has_src1
from concourse.dve_uop import DveOpSpec
from concourse.dve_table_gen import dve_ver_for
from concourse.tile import TileContext
from concourse.bass_utils import run_bass_kernel_spmd

B, T, C, H, W = 32, 8, 128, 32, 32
HW = H * W
N_CORES = 8
B_LOC = B // N_CORES
M = B_LOC * HW  # 4096
TAU = 0.5
THRESH = 1.0

_nc_cache = None


def _register_lif_step():
    name = "LIF_STEP_ANT"
    for op in dve_ops.OPS:
        if op.name == name:
            return op

    def _ref(in0, in1, s0, s1, imm2):
        m = np.where(
            np.asarray(in0, np.float32) <= np.float32(s1), in0, np.float32(0.0)
        ).astype(np.float32)
        return (m * np.float32(s0) + np.asarray(in1, np.float32)).astype(np.float32)

    body = select(Src0 <= C1, Src0, Zero) * C0 + Src1
    spec = Spec(body=body, reference=_ref)
    row = dve_ops._CUSTOM_DVE_ROW_BASE + len(dve_ops.OPS)
    ver = dve_ver_for("TRN2")
    tmp = DveOpSpec(name=name, opcode=row, uops=lower(spec, ver=ver), rd1_en=_has_src1(spec))
    op = DveOp(name, spec, subdim=False, uops_sha={ver: tmp.sha(ver)})
    dve_ops.OPS.append(op)
    dve_ops._SUB_OPCODE_FOR_NAME[name] = row
    dve_ops.CUSTOM_DVE_SPECS[name] = spec
    return op


def build_nc():
    lif_op = _register_lif_step()
    nc = bacc.Bacc("TRN2", target_bir_lowering=False)
    f32 = mybir.dt.float32
    u8 = mybir.dt.uint8
    act = mybir.ActivationFunctionType

    x = nc.dram_tensor("x", [C, T, M], f32, kind="ExternalInput")
    out = nc.dram_tensor("out", [C, T, M], u8, kind="ExternalOutput")

    with TileContext(nc) as tc:
        with (
            tc.tile_pool(name="xp", bufs=T) as xp,
            tc.tile_pool(name="up", bufs=3) as up,
            tc.tile_pool(name="sp", bufs=3) as sp,
            tc.tile_pool(name="cp", bufs=1) as cp,
        ):
            negone = cp.tile([C, 1], f32, tag="negone")
            nc.gpsimd.memset(negone[:], -1.0)

            xt = []
            with tc.high_priority():
                for t in range(T):
                    tile = xp.tile([C, M], f32, tag="x")
                    # alternate the two HWDGE rings so per-transfer completion
                    # tails overlap instead of serializing on one FIFO
                    eng = nc.sync if t % 2 == 0 else nc.scalar
                    eng.dma_start(out=tile[:], in_=x[:, t, :])
                    xt.append(tile)

            u_prev = None
            for t in range(T):
                if t == 0:
                    u = xt[0]
                else:
                    u = up.tile([C, M], f32, tag="u")
                    nc.vector._custom_dve(
                        lif_op, out=u[:], in0=u_prev[:], in1=xt[t][:],
                        s0=TAU, s1=THRESH,
                    )
                s = sp.tile([C, M], u8, tag="s")
                nc.scalar.activation(s[:], u[:], act.Sign, bias=negone[:])
                # outputs alternate rings like the inputs (4+4); each ring's
                # FIFO drains its outputs after its inputs, so reads keep
                # full bandwidth and the rings stay balanced (10 MB each)
                eng = nc.sync if t % 2 == 0 else nc.scalar
                eng.dma_start(out=out[:, t, :], in_=s[:])
                u_prev = u
    nc.compile()
    return nc


def make_in_maps(x: np.ndarray) -> list[dict]:
    # x [B,T,C,H,W] -> per core [C, T, B_loc*HW]
    xs = np.ascontiguousarray(x).reshape(B, T, C, HW)
    return [
        {
            "x": np.ascontiguousarray(
                xs[i * B_LOC : (i + 1) * B_LOC].transpose(2, 1, 0, 3)
            ).reshape(C, T, M)
        }
        for i in range(N_CORES)
    ]


def kernel(x: np.ndarray) -> np.ndarray:
    global _nc_cache
    if _nc_cache is None:
        _nc_cache = build_nc()
    res = run_bass_kernel_spmd(_nc_cache, make_in_maps(x), list(range(N_CORES)))
    # out[c, t, b_loc*HW] -> [b, t, c, hw]
    parts = [
        res.results[i]["out"].reshape(C, T, B_LOC, HW).transpose(2, 1, 0, 3)
        for i in range(N_CORES)
    ]
    full = np.concatenate(parts, axis=0)
    return full.reshape(B, T, C, H, W).astype(np.float32)


# revision 6
# speedup vs baseline: 1.2660x; 1.0665x over previous
"""LIF spike kernel for Trainium2 (Bass/Tile), data-parallel over batch on 8 cores.

Host layout per core: x_core [C=128, T=8, M=B_loc*HW=4096] f32, so each
timestep is one [128, 4096] tile (16 KB per partition, 2 MB per DMA).

State is u_t (pre-reset membrane); the hard reset folds into the next step:
  u_t     = select(u_{t-1} <= 1, u_{t-1}, 0) * 0.5 + x_t   (custom DVE op, 1 op/step)
  spike_t = sign(u_t - 1) saturated to uint8               (ACT engine, exact {0,1})

sign->u8 saturates negatives to 0 (HW-verified), so spike == (u > 1) exactly,
including u == 1 (sign(0) = 0). All arithmetic is bit-exact fp32 vs the
reference (mult by 0.5 exact, one rounding add, select exact).

Input DMAs are issued on the sync HWDGE ring before the output DMAs so the
FIFO drains all of x first; outputs (ready early) don't steal read bandwidth.
"""

import numpy as np

import concourse.bacc as bacc
import concourse.mybir as mybir
import concourse.dve_ops as dve_ops
from concourse.dve_ops import DveOp
from concourse.dve_spec import Spec, Src0, Src1, C0, C1, Zero, select, lower, _has_src1
from concourse.dve_uop import DveOpSpec
from concourse.dve_table_gen import dve_ver_for
from concourse.tile import TileContext
from concourse.bass_utils import run_bass_kernel_spmd

B, T, C, H, W = 32, 8, 128, 32, 32
HW = H * W
N_CORES = 8
B_LOC = B // N_CORES
M = B_LOC * HW  # 4096
TAU = 0.5
THRESH = 1.0

_nc_cache = None


def _register_lif_step():
    name = "LIF_STEP_ANT"
    for op in dve_ops.OPS:
        if op.name == name:
            return op

    def _ref(in0, in1, s0, s1, imm2):
        m = np.where(
            np.asarray(in0, np.float32) <= np.float32(s1), in0, np.float32(0.0)
        ).astype(np.float32)
        return (m * np.float32(s0) + np.asarray(in1, np.float32)).astype(np.float32)

    body = select(Src0 <= C1, Src0, Zero) * C0 + Src1
    spec = Spec(body=body, reference=_ref)
    row = dve_ops._CUSTOM_DVE_ROW_BASE + len(dve_ops.OPS)
    ver = dve_ver_for("TRN2")
    tmp = DveOpSpec(name=name, opcode=row, uops=lower(spec, ver=ver), rd1_en=_has_src1(spec))
    op = DveOp(name, spec, subdim=False, uops_sha={ver: tmp.sha(ver)})
    dve_ops.OPS.append(op)
    dve_ops._SUB_OPCODE_FOR_NAME[name] = row
    dve_ops.CUSTOM_DVE_SPECS[name] = spec
    return op


def build_nc():
    lif_op = _register_lif_step()
    nc = bacc.Bacc("TRN2", target_bir_lowering=False)
    f32 = mybir.dt.float32
    u8 = mybir.dt.uint8
    act = mybir.ActivationFunctionType

    x = nc.dram_tensor("x", [C, T, M], f32, kind="ExternalInput")
    out = nc.dram_tensor("out", [C, T, M], u8, kind="ExternalOutput")

    with TileContext(nc) as tc:
        with (
            tc.tile_pool(name="xp", bufs=T) as xp,
            tc.tile_pool(name="up", bufs=3) as up,
            tc.tile_pool(name="sp", bufs=3) as sp,
            tc.tile_pool(name="cp", bufs=1) as cp,
        ):
            negone = cp.tile([C, 1], f32, tag="negone")
            nc.gpsimd.memset(negone[:], -1.0)

            xt = []
            with tc.high_priority():
                for t in range(T):
                    tile = xp.tile([C, M], f32, tag="x")
                    # alternate the two HWDGE rings so per-transfer completion
                    # tails overlap instead of serializing on one FIFO;
                    # 1 MB halves (8 KB/partition runs) DMA at a better rate
                    # than one 2 MB transfer
                    eng = nc.sync if t % 2 == 0 else nc.scalar
                    h = M // 2
                    eng.dma_start(out=tile[:, :h], in_=x[:, t, :h])
                    eng.dma_start(out=tile[:, h:], in_=x[:, t, h:])
                    xt.append(tile)

            u_prev = None
            for t in range(T):
                if t == 0:
                    u = xt[0]
                else:
                    u = up.tile([C, M], f32, tag="u")
                    nc.vector._custom_dve(
                        lif_op, out=u[:], in0=u_prev[:], in1=xt[t][:],
                        s0=TAU, s1=THRESH,
                    )
                s = sp.tile([C, M], u8, tag="s")
                nc.scalar.activation(s[:], u[:], act.Sign, bias=negone[:])
                # outputs alternate rings like the inputs (4+4); each ring's
                # FIFO drains its outputs after its inputs, so reads keep
                # full bandwidth and the rings stay balanced (10 MB each)
                eng = nc.sync if t % 2 == 0 else nc.scalar
                eng.dma_start(out=out[:, t, :], in_=s[:])
                u_prev = u
    nc.compile()
    return nc


def make_in_maps(x: np.ndarray) -> list[dict]:
    # x [B,T,C,H,W] -> per core [C, T, B_loc*HW]
    xs = np.ascontiguousarray(x).reshape(B, T, C, HW)
    return [
        {
            "x": np.ascontiguousarray(
                xs[i * B_LOC : (i + 1) * B_LOC].transpose(2, 1, 0, 3)
            ).reshape(C, T, M)
        }
        for i in range(N_CORES)
    ]


def kernel(x: np.ndarray) -> np.ndarray:
    global _nc_cache
    if _nc_cache is None:
        _nc_cache = build_nc()
    res = run_bass_kernel_spmd(_nc_cache, make_in_maps(x), list(range(N_CORES)))
    # out[c, t, b_loc*HW] -> [b, t, c, hw]
    parts = [
        res.results[i]["out"].reshape(C, T, B_LOC, HW).transpose(2, 1, 0, 3)
        for i in range(N_CORES)
    ]
    full = np.concatenate(parts, axis=0)
    return full.reshape(B, T, C, H, W).astype(np.float32)
